# revision 28
# baseline (speedup 1.0000x reference)
"""Trainium2 Bass kernel for a GNN message-passing layer.

reference semantics (jax):
    src, dst = edge_index
    messages   = silu(concat(nodes[src], edge_features) @ mw1 + mb1)    # [E, D]
    aggregated = segment_sum(messages, dst, N)                          # [N, D]
    updated    = silu(concat(nodes, aggregated) @ uw1 + ub1) @ uw2 + ub2
    out        = nodes + updated

Distribution: destination-node partition across 8 cores, no collectives.
Global 128-node dst tiles are sorted by edge count and dealt 8-at-a-time
to (core, slot) pairs so each SPMD slot's compile-time tile size is the
max over a balanced group.

Host-side work is limited to layout transforms of inputs (slicing,
padding, permutation/gather of input rows into slot order, per-tile
128x128 block transposes, fp8/bf16 storage rounding, index tables) — no
float arithmetic.

Slot layout (identity scatter): within a dst tile, an edge with dst
offset j is placed at position j of one of kid[t] "identity" edge tiles
(so the scatter matrix is a constant identity); edges beyond the per-
offset identity capacity go to kov[t] overflow edge tiles scattered via
a one-hot built on DVE from dst offsets. Identity-region pad slots
contribute silu(mb1) each; a rank-1 matmul (silu(mb1) x -padcount)
cancels them exactly.

Device pipeline per core, per (8-edge-tile) PSUM chunk, software-
pipelined 2 deep so PE never waits on the ACT/DVE roundtrip:
  1. Per tile: one contiguous DMA of the [ns^T | ef^T] fp8 stream
     ([128, 2*kt*128], source rows gathered host-side — no device
     gathers), plus a one-hot build for overflow tiles only.
  2. Per edge tile: two fp8 matmuls (lhsT=ns^T/ef^T, rhs=64*mw1 halves)
     accumulate into PSUM; one DVE scalar_tensor_tensor applies the 1/64
     descale + bias (PSUM -> SBUF bf16); one SiLU (ACT, bf16).
  3. Per edge tile: a scatter matmul (lhsT=msg, rhs=identity or one-hot)
     accumulating agg^T [d, j] in PSUM; per tile: the pad-correction
     matmul, then agg^T copied to SBUF.
  4. Update MLP groups (4 node tiles, bf16 matmuls in transposed space,
     residual on gpsimd) interleaved into the edge stream; output stays
     transposed [d, n] and the host re-layouts.

Constants are packed into two DRAM tensors (fp32 + bf16) loaded with two
DMAs so startup doesn't serialize on a dozen small transfers.
"""

import math
import sys

sys.path.insert(0, "/opt/trn_rl_repo")

import numpy as np
import ml_dtypes

import concourse.bacc as bacc
import concourse.mybir as mybir
import concourse.tile as tile
from concourse import bass_utils

P = 128
C = 8  # cores

F32 = mybir.dt.float32
BF16 = mybir.dt.bfloat16
FP8 = mybir.dt.float8e4
WSCALE = 64.0
AF = mybir.ActivationFunctionType
OP = mybir.AluOpType

NP_BF16 = ml_dtypes.bfloat16
NP_FP8 = ml_dtypes.float8_e4m3


def _trunc_bf16(a):
    """fp32 -> bf16 storage conversion (round-to-nearest-even)."""
    return np.ascontiguousarray(a, np.float32).astype(NP_BF16)


def _to_fp8(a):
    """fp32 -> fp8 e4m3 storage conversion (round-to-nearest-even)."""
    return np.ascontiguousarray(a, np.float32).astype(NP_FP8)


def _blocksT(a):
    """[B*P, D] -> [P, B*D]: per-128-row-block transpose, blocks along free dim.

    out[d, b*D + e ... ] wait: out[x, b*P + e] = a[b*P + e, x]; requires D == P.
    """
    B = a.shape[0] // P
    D = a.shape[1]
    # [B, P, D] -> [B, D, P] -> [D?, ...] place block b at cols [b*P, (b+1)*P)
    t = a.reshape(B, P, D).transpose(2, 0, 1)  # [D, B, P]
    return np.ascontiguousarray(t.reshape(D, B * P))


def _host_prep(nodes, edge_index, edge_features, ntiles_pc):
    """Bucket edges by destination node tile; build per-core slot streams."""
    N, D = nodes.shape
    E = edge_index.shape[1]
    ntiles = ntiles_pc * C

    src = edge_index[0].astype(np.int64)
    dst = edge_index[1].astype(np.int64)
    tileid = dst // P
    order = np.argsort(tileid, kind="stable")
    ds = dst[order]
    ss = src[order]
    tid_s = tileid[order]

    counts = np.bincount(tileid, minlength=ntiles)
    cpt = counts.reshape(C, ntiles_pc)
    kt = [max(1, int(math.ceil(cpt[:, t].max() / P))) for t in range(ntiles_pc)]
    offs = np.zeros(ntiles_pc + 1, np.int64)
    np.cumsum(kt, out=offs[1:])
    sumkt = int(offs[-1])
    SL = sumkt * P  # slots per core

    tile_start = np.zeros(ntiles + 1, np.int64)
    np.cumsum(counts, out=tile_start[1:])
    rank = np.arange(E, dtype=np.int64) - tile_start[tid_s]
    core = tid_s // ntiles_pc
    t_local = tid_s % ntiles_pc
    slot = offs[t_local] * P + rank  # slot within the core's stream

    nodes16 = _to_fp8(nodes)
    ef16 = _to_fp8(edge_features)

    per_core = []
    for c in range(C):
        m = core == c
        sl_c = slot[m]
        # source rows + edge rows into slot order (pads stay zero)
        ns = np.zeros((SL, D), NP_FP8)
        ns[sl_c] = nodes16[ss[m]]
        ef = np.zeros((SL, D), NP_FP8)
        ef[sl_c] = ef16[order[m]]
        dof = np.full(SL, -1.0, np.float32)
        dof[sl_c] = (ds[m] - (ds[m] // P) * P).astype(np.float32)

        nsT = _blocksT(ns)  # [P, SL]
        efT = _blocksT(ef)  # [P, SL]
        # merged stream: per tile, 4-edge-tile chunks of [ns_cw | ef_cw]
        # (identity chunks first, then overflow chunks)
        nsef = np.empty((P, 2 * SL), NP_FP8)
        for t in range(ntiles_pc):
            a = int(offs[t]) * P
            pos = 2 * a
            for k0, kN in [(0, kid[t]), (kid[t], kid[t] + kov[t])]:
                k = k0
                while k < kN:
                    cw = min(4, kN - k)
                    w = cw * P
                    nsef[:, pos : pos + w] = nsT[:, a + k * P : a + k * P + w]
                    nsef[:, pos + w : pos + 2 * w] = efT[
                        :, a + k * P : a + k * P + w
                    ]
                    pos += 2 * w
                    k += cw
        dstoffT = np.ascontiguousarray(
            _trunc_bf16(dof.reshape(sumkt, P).T)
        )  # [P, sumkt]
        per_core.append(dict(nsefT=nsef, dstoffT=dstoffT))
    return kt, per_core


def build_program(D, ntiles_pc, kid, kov, debug=False):
    """Build the SPMD Bass program (identical across cores)."""
    assert D == P
    nc = bacc.Bacc("TRN2", target_bir_lowering=False, debug=False, num_devices=C)
    NP_ = ntiles_pc * P
    kt = [kid[t] + kov[t] for t in range(ntiles_pc)]
    offs = np.zeros(ntiles_pc + 1, np.int64)
    np.cumsum(kt, out=offs[1:])
    sumkt = int(offs[-1])
    ktmax = max(kt)
    ovoffs = np.zeros(ntiles_pc + 1, np.int64)
    np.cumsum(kov, out=ovoffs[1:])
    sumkov = int(ovoffs[-1])
    kovmax = max(1, max(kov))

    d = lambda name, shape, dt=F32, kind="ExternalInput": nc.dram_tensor(
        name, shape, dt, kind=kind
    ).ap()

    nsef = d("nsefT", [P, 2 * sumkt * P], FP8)
    XF = 8 * D + 2 * D + 3
    XB = kovmax * P + P + 3 * D + max(1, sumkov)
    packF = d("packF", [P, XF])
    packB = d("packB", [P, XB], BF16)
    negpad = d("negpad", [P, NP_], BF16)
    mb16 = d("mb16", [1, D], BF16)
    ownT_d = d("own_nodesT", [P, NP_])
    out = d("out_own", [P, NP_], kind="ExternalOutput")
    aggdbg = d("aggdbg", [P, ntiles_pc * D], kind="ExternalOutput") if debug else None

    with tile.TileContext(nc) as tc:
        with (
            tc.tile_pool(name="const", bufs=1) as cp,
            tc.tile_pool(name="sb", bufs=3) as sb,
            tc.tile_pool(name="big", bufs=3) as bigp,
            tc.tile_pool(name="psum", bufs=1, space="PSUM") as pp,
            tc.tile_pool(name="psum1", bufs=1, space="PSUM") as pp1,
            tc.tile_pool(name="psum3", bufs=3, space="PSUM") as pp3,
            tc.tile_pool(name="psumO", bufs=3, space="PSUM") as ppO,
        ):
            packF_s = cp.tile([P, XF], F32, tag="packF")
            nc.sync.dma_start(out=packF_s[:], in_=packF[:])
            packB_s = cp.tile([P, XB], BF16, tag="packB")
            nc.scalar.dma_start(out=packB_s[:], in_=packB[:])
            negpad_s = cp.tile([P, NP_], BF16, tag="negpad")
            nc.scalar.dma_start(out=negpad_s[:], in_=negpad[:])
            mb16_s = cp.tile([1, D], BF16, tag="mb16")
            nc.scalar.dma_start(out=mb16_s[:], in_=mb16[:])

            mbB_s = packF_s[:, : 8 * D]
            wt_f = packF_s[:, 8 * D : 9 * D]
            wb_f = packF_s[:, 9 * D : 10 * D]
            ub1_s = packF_s[:, 10 * D : 10 * D + 1]
            ub2_s = packF_s[:, 10 * D + 1 : 10 * D + 2]
            mb1c_s = packF_s[:, 10 * D + 2 : 10 * D + 3]
            iotaB_s = packB_s[:, : kovmax * P]
            id16_s = packB_s[:, kovmax * P : kovmax * P + P]
            _b0 = kovmax * P + P
            ua_s = packB_s[:, _b0 : _b0 + D]
            ub_s = packB_s[:, _b0 + D : _b0 + 2 * D]
            uw2_s = packB_s[:, _b0 + 2 * D : _b0 + 3 * D]
            doff_s = packB_s[:, _b0 + 3 * D :]
            wdr = cp.tile([D, 2 * D], FP8, tag="wdr")
            nc.vector.tensor_scalar(
                out=wdr[:, :D], in0=wt_f, scalar1=WSCALE, scalar2=None,
                op0=OP.mult,
            )
            nc.vector.tensor_scalar(
                out=wdr[:, D:], in0=wb_f, scalar1=WSCALE, scalar2=None,
                op0=OP.mult,
            )
            wdr3 = wdr[:].rearrange("p (r n) -> p r n", r=2)
            wt_s = wdr[:, :D]
            wb_s = wdr[:, D:]
            # silu(b) column via the same ACT pathway as identity-pad slots
            zcol = cp.tile([P, 1], F32, tag="zcol")
            nc.vector.memset(zcol[:], 0)
            siluB_s = cp.tile([P, 1], F32, tag="siluB")
            nc.scalar.activation(
                out=siluB_s[:], in_=zcol[:], func=AF.Silu, bias=mb1c_s
            )
            aggT_all = cp.tile([P, ntiles_pc * D], F32, tag="aggT_all")

            # ---- stage 2: dual edge pipeline ----
            # identity chunks: weights-stationary fp8 DoubleRow matmuls give
            # pre-activations transposed [dout, e]; ACT applies descale+bias
            # via its per-partition ports; identity-matmul accumulates msgT
            # into agg^T. overflow chunks: data-stationary [e, dout] path
            # with DVE descale+bias and one-hot scatter.
            chunk_list = []
            for t in range(ntiles_pc):
                pos = 0
                segs = [(0, kid[t], "I"), (kid[t], kt[t], "O")]
                items = []
                for k0, kN, typ in segs:
                    k = k0
                    while k < kN:
                        cw = min(4, kN - k)
                        items.append((t, typ, k, cw, pos))
                        pos += 2 * cw * D
                        k += cw
                for i, it in enumerate(items):
                    chunk_list.append(it + (i == 0, i == len(items) - 1))

            state = {}  # t -> (chunk_tile, paggT, ohT)

            def produce(t, typ, k, cw, pos, first):
                if first:
                    W2 = 2 * kt[t] * D
                    ctile = bigp.tile([P, 2 * ktmax * D], FP8, tag="chunk")
                    dma = nc.sync if (t % 2 == 0) else nc.scalar
                    dma.dma_start(
                        out=ctile[:, :W2],
                        in_=nsef[
                            :, 2 * int(offs[t]) * D : 2 * int(offs[t]) * D + W2
                        ],
                    )
                    paggT = pp1.tile([P, D], F32, tag="paggT")
                    KV = kov[t]
                    if KV > 0:
                        ohT = bigp.tile([P, kovmax * P], BF16, tag="ohT")
                        a = int(ovoffs[t])
                        nc.vector.tensor_tensor(
                            out=ohT[:, : KV * P].rearrange(
                                "p (f e) -> p f e", e=P
                            ),
                            in0=doff_s[:, a : a + KV].to_broadcast([P, KV, P]),
                            in1=iotaB_s[:, : KV * P].rearrange(
                                "p (f e) -> p f e", e=P
                            ),
                            op=OP.is_equal,
                        )
                    else:
                        ohT = None
                    state[t] = (ctile, paggT, ohT)
                ctile, _, _ = state[t]
                W = cw * P
                if typ == "I":
                    pmsgT = pp3.tile([P, 4 * P], F32, tag="pmsgT")
                    nc.tensor.matmul(
                        out=pmsgT[:, :W],
                        lhsT=wdr3,
                        rhs=ctile[:, pos : pos + 2 * W].rearrange(
                            "p (r e) -> p r e", r=2
                        ),
                        start=True,
                        stop=True,
                        perf_mode=mybir.MatmulPerfMode.DoubleRow,
                    )
                    return (t, typ, k, cw, pos, pmsgT)
                pmsg = ppO.tile([P, 4 * P], F32, tag="pmsg")
                for j in range(cw):
                    reg = pmsg[:, j * P : (j + 1) * P]
                    nc.tensor.matmul(
                        out=reg,
                        lhsT=ctile[:, pos + j * D : pos + (j + 1) * D],
                        rhs=wt_s,
                        start=True,
                        stop=False,
                    )
                    nc.tensor.matmul(
                        out=reg,
                        lhsT=ctile[:, pos + (cw + j) * D : pos + (cw + j + 1) * D],
                        rhs=wb_s,
                        start=False,
                        stop=True,
                    )
                return (t, typ, k, cw, pos, pmsg)

            def consume(pend, last):
                t, typ, k0, cw, pos, pm = pend
                W = cw * P
                _, paggT, ohT = state[t]
                KI = kid[t]
                if typ == "I":
                    msgT = sb.tile([P, 4 * P], BF16, tag="msgT")
                    nc.scalar.activation(
                        out=msgT[:, :W], in_=pm[:, :W], func=AF.Silu,
                        scale=1.0 / WSCALE, bias=mb1c_s,
                    )
                    for j in range(cw):
                        k = k0 + j
                        # agg^T[:, j'] += msgT tile (identity scatter)
                        nc.tensor.matmul(
                            out=paggT[:],
                            lhsT=id16_s,
                            rhs=msgT[:, j * P : (j + 1) * P],
                            start=(k == 0),
                            stop=(last and k == kt[t] - 1),
                            skip_group_check=True,
                        )
                else:
                    mpre = sb.tile([P, 4 * P], BF16, tag="mpre")
                    nc.vector.scalar_tensor_tensor(
                        out=mpre[:, :W], in0=pm[:, :W], scalar=1.0 / WSCALE,
                        in1=mbB_s[:, :W], op0=OP.mult, op1=OP.add,
                    )
                    msg = sb.tile([P, 4 * P], BF16, tag="msg")
                    nc.scalar.activation(
                        out=msg[:, :W], in_=mpre[:, :W], func=AF.Silu
                    )
                    for j in range(cw):
                        k = k0 + j
                        nc.tensor.matmul(
                            out=paggT[:],
                            lhsT=msg[:, j * P : (j + 1) * P],
                            rhs=ohT[:, (k - KI) * P : (k - KI + 1) * P],
                            start=(k == 0),
                            stop=(last and k == kt[t] - 1),
                            skip_group_check=True,
                        )
                if last:
                    # cancel identity-region pad contributions exactly:
                    # agg[d, j] -= silu(b)[d] * padcnt[t][j]
                    nc.vector.scalar_tensor_tensor(
                        out=paggT[:],
                        in0=negpad_s[:, t * P : (t + 1) * P],
                        scalar=siluB_s[:, :1],
                        in1=paggT[:],
                        op0=OP.mult,
                        op1=OP.add,
                    )
                    nc.vector.tensor_copy(
                        out=aggT_all[:, t * D : (t + 1) * D], in_=paggT[:]
                    )

            def stage3_group(g):
                gw = min(4, ntiles_pc - g)
                W = gw * P
                ownT = sb.tile([P, 4 * P], F32, tag="ownT")
                nc.sync.dma_start(
                    out=ownT[:, :W], in_=ownT_d[:, g * P : g * P + W]
                )
                ownT16 = sb.tile([P, 4 * P], BF16, tag="ownT16")
                nc.gpsimd.tensor_copy(out=ownT16[:, :W], in_=ownT[:, :W])
                agg16 = sb.tile([P, 4 * P], BF16, tag="agg16")
                nc.gpsimd.tensor_copy(
                    out=agg16[:, :W], in_=aggT_all[:, g * D : g * D + W]
                )
                ph = pp.tile([P, 4 * P], F32, tag="ph")
                nc.tensor.matmul(
                    out=ph[:, :W], lhsT=ua_s, rhs=ownT16[:, :W],
                    start=True, stop=False,
                )
                nc.tensor.matmul(
                    out=ph[:, :W], lhsT=ub_s, rhs=agg16[:, :W],
                    start=False, stop=True,
                )
                hT = sb.tile([P, 4 * P], BF16, tag="hT")
                nc.scalar.activation(
                    out=hT[:, :W], in_=ph[:, :W], func=AF.Silu, bias=ub1_s
                )
                po = pp.tile([P, 4 * P], F32, tag="ph")
                nc.tensor.matmul(
                    out=po[:, :W], lhsT=uw2_s, rhs=hT[:, :W], start=True, stop=True
                )
                oT = sb.tile([P, 4 * P], F32, tag="oT")
                nc.scalar.activation(
                    out=oT[:, :W], in_=po[:, :W], func=AF.Identity, bias=ub2_s
                )
                nc.gpsimd.tensor_tensor(
                    out=oT[:, :W], in0=oT[:, :W], in1=ownT[:, :W], op=OP.add
                )
                nc.sync.dma_start(
                    out=out[:, g * P : g * P + W], in_=oT[:, :W]
                )

            # ---- main loop: 2-deep chunk pipeline, stage-3 interleaved ----
            from collections import deque

            pending = deque()
            done_tiles = 0

            def after_consume(t, last):
                nonlocal done_tiles
                if last:
                    done_tiles += 1
                    if done_tiles % 4 == 0:
                        stage3_group(done_tiles - 4)

            for t, typ, k, cw, pos, first, last in chunk_list:
                cur = produce(t, typ, k, cw, pos, first)
                pending.append((cur, last))
                if len(pending) > 2:
                    (pc, pl) = pending.popleft()
                    consume(pc, pl)
                    after_consume(pc[0], pl)
            while pending:
                (pc, pl) = pending.popleft()
                consume(pc, pl)
                after_consume(pc[0], pl)
            for g in range(4 * (done_tiles // 4), ntiles_pc, 4):
                stage3_group(g)
            if debug:
                nc.sync.dma_start(out=aggdbg[:], in_=aggT_all[:])

    nc.compile()
    return nc


def _run(nc, in_maps, trace=False):
    return bass_utils.run_bass_kernel_spmd(
        nc, in_maps, core_ids=list(range(C)), trace=trace
    )


def make_in_maps(nodes, edge_index, edge_features, mw1, mb1, uw1, ub1, uw2, ub2,
                 ntiles_pc):
    N, D = nodes.shape
    NP_ = ntiles_pc * P
    N2 = NP_ * C
    kt, per_core = _host_prep(nodes, edge_index, edge_features, ntiles_pc)

    nodes_pad = np.zeros((N2, D), np.float32)
    nodes_pad[:N] = nodes
    kovmax = max(1, max(kov))
    iotaB = np.broadcast_to(
        np.tile(np.arange(P, dtype=np.float32), kovmax), (P, kovmax * P)
    ).astype(np.float32)
    packB_common = np.concatenate(
        [
            _trunc_bf16(iotaB),
            _trunc_bf16(np.eye(P, dtype=np.float32)),
            _trunc_bf16(uw1[:D]),
            _trunc_bf16(uw1[D:]),
            _trunc_bf16(uw2),
        ],
        axis=1,
    )

    packF = np.concatenate(
        [
            np.broadcast_to(np.tile(mb1.astype(np.float32), 8), (P, 8 * D)),
            np.ascontiguousarray(mw1[:D], np.float32).T.T,
            np.ascontiguousarray(mw1[D:], np.float32),
            ub1.reshape(D, 1).astype(np.float32),
            ub2.reshape(D, 1).astype(np.float32),
            mb1.reshape(D, 1).astype(np.float32),
        ],
        axis=1,
    ).astype(np.float32)
    shared = dict(
        packF=np.ascontiguousarray(packF),
        mb16=_trunc_bf16(mb1.reshape(1, D)),
    )
    in_maps = []
    for c in range(C):
        m = dict(shared)
        own = nodes_pad[c * NP_ : (c + 1) * NP_]  # [NP_, D]
        m["own_nodesT"] = np.ascontiguousarray(own.T)  # [P(d), NP_]
        m["nsefT"] = per_core[c]["nsefT"]
        m["dstoffT"] = per_core[c]["dstoffT"]
        in_maps.append(m)
    return kt, in_maps


def kernel(nodes, edge_index, edge_features, mw1, mb1, uw1, ub1, uw2, ub2):
    nodes = np.asarray(nodes, np.float32)
    edge_index = np.asarray(edge_index, np.int32)
    edge_features = np.asarray(edge_features, np.float32)
    N, D = nodes.shape
    ntiles_pc = math.ceil(N / (C * P))
    kt, in_maps = make_in_maps(
        nodes, edge_index, edge_features, mw1, mb1, uw1, ub1, uw2, ub2, ntiles_pc
    )
    nc = build_program(D, ntiles_pc, kt)
    res = _run(nc, in_maps)
    NP_ = ntiles_pc * P
    # out_own is [P(d? no: partition = n%128), ntiles*D] -> rows
    outs = [np.ascontiguousarray(res.results[c]["out_own"].T) for c in range(C)]
    out = np.concatenate(outs, axis=0)
    return out[:N].astype(np.float32)


if __name__ == "__main__":
    rng = np.random.default_rng(0)
    N, E, D = 4096, 16384, 128
    nodes = rng.standard_normal((N, D), dtype=np.float32)
    edge_index = rng.integers(0, N, (2, E)).astype(np.int32)
    ef = rng.standard_normal((E, D), dtype=np.float32)
    s2, s1 = 1 / np.sqrt(2 * D), 1 / np.sqrt(D)
    mw1 = rng.uniform(-s2, s2, (2 * D, D)).astype(np.float32)
    mb1 = rng.uniform(-s2, s2, D).astype(np.float32)
    uw1 = rng.uniform(-s2, s2, (2 * D, D)).astype(np.float32)
    ub1 = rng.uniform(-s2, s2, D).astype(np.float32)
    uw2 = rng.uniform(-s1, s1, (D, D)).astype(np.float32)
    ub2 = rng.uniform(-s1, s1, D).astype(np.float32)

    def silu(x):
        return x / (1 + np.exp(-x))

    def ref():
        src, dst = edge_index
        msg = silu(np.concatenate([nodes[src], ef], 1) @ mw1 + mb1)
        agg = np.zeros((N, D), np.float32)
        np.add.at(agg, dst, msg)
        upd = silu(np.concatenate([nodes, agg], 1) @ uw1 + ub1) @ uw2 + ub2
        return nodes + upd
    out = kernel(nodes, edge_index, ef, mw1, mb1, uw1, ub1, uw2, ub2)
    exp = ref()
    err = np.abs(out - exp).max() / np.abs(exp).max()
    print("tiny rel err:", err)


# revision 30
# speedup vs baseline: 1.1716x; 1.1716x over previous
"""Trainium2 Bass kernel for a GNN message-passing layer.

reference semantics (jax):
    src, dst = edge_index
    messages   = silu(concat(nodes[src], edge_features) @ mw1 + mb1)    # [E, D]
    aggregated = segment_sum(messages, dst, N)                          # [N, D]
    updated    = silu(concat(nodes, aggregated) @ uw1 + ub1) @ uw2 + ub2
    out        = nodes + updated

Distribution: destination-node partition across 8 cores. Each core owns a
contiguous 1/8 slice of the (padded) node range, aggregates exactly the
edges landing in its slice, and runs the update MLP on its slice. No
collectives.

Host-side work is limited to layout transforms of inputs (slicing,
padding, permutation/gather of input rows into slot order, per-tile
128x128 block transposes, bf16 byte-truncation, index tables) — no float
arithmetic.

Slot layout: edges are bucketed by destination node tile (128 dst nodes
per tile). Local tile t owns kt[t] edge tiles of 128 slots (kt = max
over cores, a compile-time constant); leftover slots are pads with
dst-offset -1 so their junk messages scatter with weight 0. The host
streams, per edge slot, BOTH the source-node row nodes[src] and the
edge-feature row (pre-transposed per 128-tile, bf16), so the device does
no gathers at all.

Device pipeline per core, per local node tile t:
  1. One contiguous DMA of the [ns^T | ef^T] chunk (bf16, [128, 2*kt*128]).
  2. Per 4-edge-tile chunk: per edge tile a 1-partition ones-matmul adds
     the message bias into PSUM (start=True), then ns/ef matmuls
     accumulate; one SiLU (PSUM -> SBUF bf16); one wide DVE is_equal
     builds all 4 one-hots at once (broadcast dst-offset columns vs a
     tiled iota).
  3. Per edge tile: a scatter matmul (lhsT=msg, rhs=one-hot)
     accumulating agg^T [d, j] in PSUM.
  Chunks are software-pipelined: chunk i's matmuls are emitted before
  chunk i-1's silu/scatter consumers so the PE stream never waits on the
  ACT/DVE roundtrip.
  4. Update MLP in transposed space (4 node tiles per group), residual,
     transpose back, store (partition-major output, host re-layouts).
"""

import math
import sys

sys.path.insert(0, "/opt/trn_rl_repo")

import numpy as np
import ml_dtypes

import concourse.bacc as bacc
import concourse.mybir as mybir
import concourse.tile as tile
from concourse import bass_utils

P = 128
C = 8  # cores
ONEHOT_GPSIMD = False  # Pool engine fails ISA check for is_equal tensor_tensor

F32 = mybir.dt.float32
BF16 = mybir.dt.bfloat16
FP8 = mybir.dt.float8e4
WSCALE = 64.0
AF = mybir.ActivationFunctionType
OP = mybir.AluOpType

NP_BF16 = ml_dtypes.bfloat16
NP_FP8 = ml_dtypes.float8_e4m3


def _trunc_bf16(a):
    """fp32 -> bf16 storage conversion (round-to-nearest-even)."""
    return np.ascontiguousarray(a, np.float32).astype(NP_BF16)


def _to_fp8(a):
    """fp32 -> fp8 e4m3 storage conversion (round-to-nearest-even)."""
    return np.ascontiguousarray(a, np.float32).astype(NP_FP8)


def _blocksT(a):
    """[B*P, D] -> [P, B*D]: per-128-row-block transpose, blocks along free dim.

    out[d, b*D + e ... ] wait: out[x, b*P + e] = a[b*P + e, x]; requires D == P.
    """
    B = a.shape[0] // P
    D = a.shape[1]
    # [B, P, D] -> [B, D, P] -> [D?, ...] place block b at cols [b*P, (b+1)*P)
    t = a.reshape(B, P, D).transpose(2, 0, 1)  # [D, B, P]
    return np.ascontiguousarray(t.reshape(D, B * P))


def _host_prep(nodes, edge_index, edge_features, ntiles_pc):
    """Bucket edges by destination node tile; build per-core slot streams."""
    N, D = nodes.shape
    E = edge_index.shape[1]
    ntiles = ntiles_pc * C

    src = edge_index[0].astype(np.int64)
    dst = edge_index[1].astype(np.int64)
    tileid = dst // P
    order = np.argsort(tileid, kind="stable")
    ds = dst[order]
    ss = src[order]
    tid_s = tileid[order]

    counts = np.bincount(tileid, minlength=ntiles)
    cpt = counts.reshape(C, ntiles_pc)
    kt = [max(1, int(math.ceil(cpt[:, t].max() / P))) for t in range(ntiles_pc)]
    offs = np.zeros(ntiles_pc + 1, np.int64)
    np.cumsum(kt, out=offs[1:])
    sumkt = int(offs[-1])
    SL = sumkt * P  # slots per core

    tile_start = np.zeros(ntiles + 1, np.int64)
    np.cumsum(counts, out=tile_start[1:])
    rank = np.arange(E, dtype=np.int64) - tile_start[tid_s]
    core = tid_s // ntiles_pc
    t_local = tid_s % ntiles_pc
    slot = offs[t_local] * P + rank  # slot within the core's stream

    nodes16 = _to_fp8(nodes)
    ef16 = _to_fp8(edge_features)

    per_core = []
    for c in range(C):
        m = core == c
        sl_c = slot[m]
        # source rows + edge rows into slot order (pads stay zero)
        ns = np.zeros((SL, D), NP_FP8)
        ns[sl_c] = nodes16[ss[m]]
        ef = np.zeros((SL, D), NP_FP8)
        ef[sl_c] = ef16[order[m]]
        dof = np.full(SL, -1.0, np.float32)
        dof[sl_c] = (ds[m] - (ds[m] // P) * P).astype(np.float32)

        nsT = _blocksT(ns)  # [P, SL]
        efT = _blocksT(ef)  # [P, SL]
        # merged stream: per tile t, kt[t] ns-tiles then kt[t] ef-tiles
        nsef = np.empty((P, 2 * SL), NP_FP8)
        for t in range(ntiles_pc):
            a, b = int(offs[t]) * P, int(offs[t + 1]) * P
            w = b - a
            nsef[:, 2 * a : 2 * a + w] = nsT[:, a:b]
            nsef[:, 2 * a + w : 2 * b] = efT[:, a:b]
        dstoffT = np.ascontiguousarray(
            _trunc_bf16(dof.reshape(sumkt, P).T)
        )  # [P, sumkt]
        per_core.append(dict(nsefT=nsef, dstoffT=dstoffT))
    return kt, per_core


def build_program(D, ntiles_pc, kid, kov, debug=False):
    """Build the SPMD Bass program (identical across cores)."""
    assert D == P
    nc = bacc.Bacc("TRN2", target_bir_lowering=False, debug=False, num_devices=C)
    NP_ = ntiles_pc * P
    kt = [kid[t] + kov[t] for t in range(ntiles_pc)]
    offs = np.zeros(ntiles_pc + 1, np.int64)
    np.cumsum(kt, out=offs[1:])
    sumkt = int(offs[-1])
    ktmax = max(kt)
    ovoffs = np.zeros(ntiles_pc + 1, np.int64)
    np.cumsum(kov, out=ovoffs[1:])
    sumkov = int(ovoffs[-1])
    kovmax = max(1, max(kov))

    d = lambda name, shape, dt=F32, kind="ExternalInput": nc.dram_tensor(
        name, shape, dt, kind=kind
    ).ap()

    nsef = d("nsefT", [P, 2 * sumkt * P], FP8)
    XF = 8 * D + 2 * D + 3
    XB = kovmax * P + P + 3 * D + max(1, sumkov)
    packF = d("packF", [P, XF])
    packB = d("packB", [P, XB], BF16)
    negpad = d("negpad", [P, NP_], BF16)
    ownT_d = d("own_nodesT", [P, NP_])
    out = d("out_own", [P, NP_], kind="ExternalOutput")
    aggdbg = d("aggdbg", [P, ntiles_pc * D], kind="ExternalOutput") if debug else None

    with tile.TileContext(nc) as tc:
        with (
            tc.tile_pool(name="const", bufs=1) as cp,
            tc.tile_pool(name="sb", bufs=4) as sb,
            tc.tile_pool(name="big", bufs=4) as bigp,
            tc.tile_pool(name="psum", bufs=1, space="PSUM") as pp,
            tc.tile_pool(name="psum1", bufs=1, space="PSUM") as pp1,
            tc.tile_pool(name="psum3", bufs=3, space="PSUM") as pp3,
        ):
            packF_s = cp.tile([P, XF], F32, tag="packF")
            nc.sync.dma_start(out=packF_s[:], in_=packF[:])
            packB_s = cp.tile([P, XB], BF16, tag="packB")
            nc.scalar.dma_start(out=packB_s[:], in_=packB[:])
            negpad_s = cp.tile([P, NP_], BF16, tag="negpad")
            nc.scalar.dma_start(out=negpad_s[:], in_=negpad[:])

            mbB_s = packF_s[:, : 8 * D]
            wt_f = packF_s[:, 8 * D : 9 * D]
            wb_f = packF_s[:, 9 * D : 10 * D]
            ub1_s = packF_s[:, 10 * D : 10 * D + 1]
            ub2_s = packF_s[:, 10 * D + 1 : 10 * D + 2]
            mb1c_s = packF_s[:, 10 * D + 2 : 10 * D + 3]
            iotaB_s = packB_s[:, : kovmax * P]
            id16_s = packB_s[:, kovmax * P : kovmax * P + P]
            _b0 = kovmax * P + P
            ua_s = packB_s[:, _b0 : _b0 + D]
            ub_s = packB_s[:, _b0 + D : _b0 + 2 * D]
            uw2_s = packB_s[:, _b0 + 2 * D : _b0 + 3 * D]
            doff_s = packB_s[:, _b0 + 3 * D :]
            wt_s = cp.tile([D, D], FP8, tag="wt8")
            wb_s = cp.tile([D, D], FP8, tag="wb8")
            nc.vector.tensor_scalar(
                out=wt_s[:], in0=wt_f, scalar1=WSCALE, scalar2=None, op0=OP.mult
            )
            nc.vector.tensor_scalar(
                out=wb_s[:], in0=wb_f, scalar1=WSCALE, scalar2=None, op0=OP.mult
            )
            zcol = cp.tile([P, 1], F32, tag="zcol")
            nc.vector.memset(zcol[:], 0)
            siluB_s = cp.tile([P, 1], F32, tag="siluB")
            nc.scalar.activation(
                out=siluB_s[:], in_=zcol[:], func=AF.Silu, bias=mb1c_s
            )
            aggT_all = cp.tile([P, ntiles_pc * D], F32, tag="aggT_all")

            # ---- stage 2: edge pipeline (software-pipelined by 1 chunk) ----
            CH = 8  # edge tiles per PSUM chunk (2 banks)
            chunks = []
            for t in range(ntiles_pc):
                nch = math.ceil(kt[t] / CH)
                for ci in range(nch):
                    chunks.append((t, ci, ci == 0, ci == nch - 1))

            state = {}  # t -> (chunk_tile, paggT, ohT)

            def produce(t, ci, first):
                KT = kt[t]
                if first:
                    W2 = 2 * KT * D
                    ctile = bigp.tile([P, 2 * ktmax * D], FP8, tag="chunk")
                    base = 2 * int(offs[t]) * D
                    h = (W2 // 2 + P - 1) // P * P
                    nc.sync.dma_start(
                        out=ctile[:, :h], in_=nsef[:, base : base + h]
                    )
                    nc.scalar.dma_start(
                        out=ctile[:, h:W2], in_=nsef[:, base + h : base + W2]
                    )
                    paggT = pp1.tile([P, D], F32, tag="paggT")
                    KV = kov[t]
                    if KV > 0:
                        ohT = bigp.tile([P, kovmax * P], BF16, tag="ohT")
                        a = int(ovoffs[t])
                        nc.vector.tensor_tensor(
                            out=ohT[:, : KV * P].rearrange(
                                "p (f e) -> p f e", e=P
                            ),
                            in0=doff_s[:, a : a + KV].to_broadcast([P, KV, P]),
                            in1=iotaB_s[:, : KV * P].rearrange(
                                "p (f e) -> p f e", e=P
                            ),
                            op=OP.is_equal,
                        )
                    else:
                        ohT = None
                    state[t] = (ctile, paggT, ohT)
                ctile, _, _ = state[t]
                k0 = ci * CH
                cw = min(CH, KT - k0)
                pmsg = pp3.tile([P, CH * P], F32, tag="pmsg")
                for j in range(cw):
                    k = k0 + j
                    reg = pmsg[:, j * P : (j + 1) * P]
                    nc.tensor.matmul(
                        out=reg, lhsT=ctile[:, k * D : (k + 1) * D], rhs=wt_s[:],
                        start=True, stop=False,
                    )
                    nc.tensor.matmul(
                        out=reg,
                        lhsT=ctile[:, (KT + k) * D : (KT + k + 1) * D],
                        rhs=wb_s[:],
                        start=False, stop=True,
                    )
                return (t, ci, cw, pmsg)

            def consume(pend, last):
                t, ci, cw, pmsg = pend
                KT = kt[t]
                k0 = ci * CH
                W = cw * P
                _, paggT, ohT = state[t]
                mpre = sb.tile([P, CH * P], BF16, tag="mpre")
                nc.vector.scalar_tensor_tensor(
                    out=mpre[:, :W], in0=pmsg[:, :W], scalar=1.0 / WSCALE,
                    in1=mbB_s[:, :W], op0=OP.mult, op1=OP.add,
                )
                msg = sb.tile([P, CH * P], BF16, tag="msg")
                nc.scalar.activation(out=msg[:, :W], in_=mpre[:, :W], func=AF.Silu)
                KI = kid[t]
                for j in range(cw):
                    k = k0 + j
                    # aggT[d, j] += sum_e msg[e, d] * scatter[e, j]
                    rhs = (
                        id16_s
                        if k < KI
                        else ohT[:, (k - KI) * P : (k - KI + 1) * P]
                    )
                    nc.tensor.matmul(
                        out=paggT[:],
                        lhsT=msg[:, j * P : (j + 1) * P],
                        rhs=rhs,
                        start=(k == 0),
                        stop=(last and k == KT - 1),
                        skip_group_check=True,
                    )
                if last:
                    # copy agg^T out while cancelling identity-pad
                    # contributions exactly: agg -= silu(b)[d]*padcnt[t][j]
                    nc.vector.scalar_tensor_tensor(
                        out=aggT_all[:, t * D : (t + 1) * D],
                        in0=negpad_s[:, t * P : (t + 1) * P],
                        scalar=siluB_s[:, :1],
                        in1=paggT[:],
                        op0=OP.mult,
                        op1=OP.add,
                    )

            def stage3_group(g):
                gw = min(4, ntiles_pc - g)
                W = gw * P
                ownT = sb.tile([P, 4 * P], F32, tag="ownT")
                nc.sync.dma_start(
                    out=ownT[:, :W], in_=ownT_d[:, g * P : g * P + W]
                )
                ownT16 = sb.tile([P, 4 * P], BF16, tag="ownT16")
                nc.gpsimd.tensor_copy(out=ownT16[:, :W], in_=ownT[:, :W])
                agg16 = sb.tile([P, 4 * P], BF16, tag="agg16")
                nc.gpsimd.tensor_copy(
                    out=agg16[:, :W], in_=aggT_all[:, g * D : g * D + W]
                )
                ph = pp.tile([P, 4 * P], F32, tag="ph")
                nc.tensor.matmul(
                    out=ph[:, :W], lhsT=ua_s, rhs=ownT16[:, :W],
                    start=True, stop=False,
                )
                nc.tensor.matmul(
                    out=ph[:, :W], lhsT=ub_s, rhs=agg16[:, :W],
                    start=False, stop=True,
                )
                hT = sb.tile([P, 4 * P], BF16, tag="hT")
                nc.scalar.activation(
                    out=hT[:, :W], in_=ph[:, :W], func=AF.Silu, bias=ub1_s
                )
                po = pp.tile([P, 4 * P], F32, tag="ph")
                nc.tensor.matmul(
                    out=po[:, :W], lhsT=uw2_s, rhs=hT[:, :W], start=True, stop=True
                )
                oT = sb.tile([P, 4 * P], F32, tag="oT")
                nc.scalar.activation(
                    out=oT[:, :W], in_=po[:, :W], func=AF.Identity, bias=ub2_s
                )
                nc.gpsimd.tensor_tensor(
                    out=oT[:, :W], in0=oT[:, :W], in1=ownT[:, :W], op=OP.add
                )
                nc.sync.dma_start(
                    out=out[:, g * P : g * P + W], in_=oT[:, :W]
                )

            # ---- main loop: 2-deep chunk pipeline, stage-3 interleaved ----
            from collections import deque

            pending = deque()
            done_tiles = 0

            def after_consume(t, last):
                nonlocal done_tiles
                if last:
                    done_tiles += 1
                    if done_tiles % 4 == 0:
                        stage3_group(done_tiles - 4)

            for t, ci, first, last in chunks:
                cur = produce(t, ci, first)
                pending.append((cur, last))
                if len(pending) > 2:
                    (pc, pl) = pending.popleft()
                    consume(pc, pl)
                    after_consume(pc[0], pl)
            while pending:
                (pc, pl) = pending.popleft()
                consume(pc, pl)
                after_consume(pc[0], pl)
            for g in range(4 * (done_tiles // 4), ntiles_pc, 4):
                stage3_group(g)
            if debug:
                nc.sync.dma_start(out=aggdbg[:], in_=aggT_all[:])

    nc.compile()
    return nc


def _run(nc, in_maps, trace=False):
    return bass_utils.run_bass_kernel_spmd(
        nc, in_maps, core_ids=list(range(C)), trace=trace
    )


def make_in_maps(nodes, edge_index, edge_features, mw1, mb1, uw1, ub1, uw2, ub2,
                 ntiles_pc):
    N, D = nodes.shape
    NP_ = ntiles_pc * P
    N2 = NP_ * C
    kt, per_core = _host_prep(nodes, edge_index, edge_features, ntiles_pc)

    nodes_pad = np.zeros((N2, D), np.float32)
    nodes_pad[:N] = nodes
    kovmax = max(1, max(kov))
    iotaB = np.broadcast_to(
        np.tile(np.arange(P, dtype=np.float32), kovmax), (P, kovmax * P)
    ).astype(np.float32)
    packB_common = np.concatenate(
        [
            _trunc_bf16(iotaB),
            _trunc_bf16(np.eye(P, dtype=np.float32)),
            _trunc_bf16(uw1[:D]),
            _trunc_bf16(uw1[D:]),
            _trunc_bf16(uw2),
        ],
        axis=1,
    )

    packF = np.concatenate(
        [
            np.broadcast_to(np.tile(mb1.astype(np.float32), 8), (P, 8 * D)),
            np.ascontiguousarray(mw1[:D], np.float32).T.T,
            np.ascontiguousarray(mw1[D:], np.float32),
            ub1.reshape(D, 1).astype(np.float32),
            ub2.reshape(D, 1).astype(np.float32),
            _trunc_bf16(mb1.reshape(D, 1)).astype(np.float32),
        ],
        axis=1,
    ).astype(np.float32)
    shared = dict(
        packF=np.ascontiguousarray(packF),
    )
    in_maps = []
    for c in range(C):
        m = dict(shared)
        own = nodes_pad[c * NP_ : (c + 1) * NP_]  # [NP_, D]
        m["own_nodesT"] = np.ascontiguousarray(own.T)  # [P(d), NP_]
        m["nsefT"] = per_core[c]["nsefT"]
        m["dstoffT"] = per_core[c]["dstoffT"]
        in_maps.append(m)
    return kt, in_maps


def kernel(nodes, edge_index, edge_features, mw1, mb1, uw1, ub1, uw2, ub2):
    nodes = np.asarray(nodes, np.float32)
    edge_index = np.asarray(edge_index, np.int32)
    edge_features = np.asarray(edge_features, np.float32)
    N, D = nodes.shape
    ntiles_pc = math.ceil(N / (C * P))
    kt, in_maps = make_in_maps(
        nodes, edge_index, edge_features, mw1, mb1, uw1, ub1, uw2, ub2, ntiles_pc
    )
    nc = build_program(D, ntiles_pc, kt)
    res = _run(nc, in_maps)
    NP_ = ntiles_pc * P
    # out_own is [P(d? no: partition = n%128), ntiles*D] -> rows
    outs = [np.ascontiguousarray(res.results[c]["out_own"].T) for c in range(C)]
    out = np.concatenate(outs, axis=0)
    return out[:N].astype(np.float32)


if __name__ == "__main__":
    rng = np.random.default_rng(0)
    N, E, D = 4096, 16384, 128
    nodes = rng.standard_normal((N, D), dtype=np.float32)
    edge_index = rng.integers(0, N, (2, E)).astype(np.int32)
    ef = rng.standard_normal((E, D), dtype=np.float32)
    s2, s1 = 1 / np.sqrt(2 * D), 1 / np.sqrt(D)
    mw1 = rng.uniform(-s2, s2, (2 * D, D)).astype(np.float32)
    mb1 = rng.uniform(-s2, s2, D).astype(np.float32)
    uw1 = rng.uniform(-s2, s2, (2 * D, D)).astype(np.float32)
    ub1 = rng.uniform(-s2, s2, D).astype(np.float32)
    uw2 = rng.uniform(-s1, s1, (D, D)).astype(np.float32)
    ub2 = rng.uniform(-s1, s1, D).astype(np.float32)

    def silu(x):
        return x / (1 + np.exp(-x))

    def ref():
        src, dst = edge_index
        msg = silu(np.concatenate([nodes[src], ef], 1) @ mw1 + mb1)
        agg = np.zeros((N, D), np.float32)
        np.add.at(agg, dst, msg)
        upd = silu(np.concatenate([nodes, agg], 1) @ uw1 + ub1) @ uw2 + ub2
        return nodes + upd
    out = kernel(nodes, edge_index, ef, mw1, mb1, uw1, ub1, uw2, ub2)
    exp = ref()
    err = np.abs(out - exp).max() / np.abs(exp).max()
    print("tiny rel err:", err)


# revision 31
# speedup vs baseline: 1.2191x; 1.0406x over previous
"""Trainium2 Bass kernel for a GNN message-passing layer.

reference semantics (jax):
    src, dst = edge_index
    messages   = silu(concat(nodes[src], edge_features) @ mw1 + mb1)    # [E, D]
    aggregated = segment_sum(messages, dst, N)                          # [N, D]
    updated    = silu(concat(nodes, aggregated) @ uw1 + ub1) @ uw2 + ub2
    out        = nodes + updated

Distribution: destination-node partition across 8 cores. Each core owns a
contiguous 1/8 slice of the (padded) node range, aggregates exactly the
edges landing in its slice, and runs the update MLP on its slice. No
collectives.

Host-side work is limited to layout transforms of inputs (slicing,
padding, permutation/gather of input rows into slot order, per-tile
128x128 block transposes, bf16 byte-truncation, index tables) — no float
arithmetic.

Slot layout: edges are bucketed by destination node tile (128 dst nodes
per tile). Local tile t owns kt[t] edge tiles of 128 slots (kt = max
over cores, a compile-time constant); leftover slots are pads with
dst-offset -1 so their junk messages scatter with weight 0. The host
streams, per edge slot, BOTH the source-node row nodes[src] and the
edge-feature row (pre-transposed per 128-tile, bf16), so the device does
no gathers at all.

Device pipeline per core, per local node tile t:
  1. One contiguous DMA of the [ns^T | ef^T] chunk (bf16, [128, 2*kt*128]).
  2. Per 4-edge-tile chunk: per edge tile a 1-partition ones-matmul adds
     the message bias into PSUM (start=True), then ns/ef matmuls
     accumulate; one SiLU (PSUM -> SBUF bf16); one wide DVE is_equal
     builds all 4 one-hots at once (broadcast dst-offset columns vs a
     tiled iota).
  3. Per edge tile: a scatter matmul (lhsT=msg, rhs=one-hot)
     accumulating agg^T [d, j] in PSUM.
  Chunks are software-pipelined: chunk i's matmuls are emitted before
  chunk i-1's silu/scatter consumers so the PE stream never waits on the
  ACT/DVE roundtrip.
  4. Update MLP in transposed space (4 node tiles per group), residual,
     transpose back, store (partition-major output, host re-layouts).
"""

import math
import sys

sys.path.insert(0, "/opt/trn_rl_repo")

import numpy as np
import ml_dtypes

import concourse.bacc as bacc
import concourse.mybir as mybir
import concourse.tile as tile
from concourse import bass_utils

P = 128
C = 8  # cores
ONEHOT_GPSIMD = False  # Pool engine fails ISA check for is_equal tensor_tensor

F32 = mybir.dt.float32
BF16 = mybir.dt.bfloat16
FP8 = mybir.dt.float8e4
WSCALE = 64.0
AF = mybir.ActivationFunctionType
OP = mybir.AluOpType

NP_BF16 = ml_dtypes.bfloat16
NP_FP8 = ml_dtypes.float8_e4m3


def _trunc_bf16(a):
    """fp32 -> bf16 storage conversion (round-to-nearest-even)."""
    return np.ascontiguousarray(a, np.float32).astype(NP_BF16)


def _to_fp8(a):
    """fp32 -> fp8 e4m3 storage conversion (round-to-nearest-even)."""
    return np.ascontiguousarray(a, np.float32).astype(NP_FP8)


def _blocksT(a):
    """[B*P, D] -> [P, B*D]: per-128-row-block transpose, blocks along free dim.

    out[d, b*D + e ... ] wait: out[x, b*P + e] = a[b*P + e, x]; requires D == P.
    """
    B = a.shape[0] // P
    D = a.shape[1]
    # [B, P, D] -> [B, D, P] -> [D?, ...] place block b at cols [b*P, (b+1)*P)
    t = a.reshape(B, P, D).transpose(2, 0, 1)  # [D, B, P]
    return np.ascontiguousarray(t.reshape(D, B * P))


def _host_prep(nodes, edge_index, edge_features, ntiles_pc):
    """Bucket edges by destination node tile; build per-core slot streams."""
    N, D = nodes.shape
    E = edge_index.shape[1]
    ntiles = ntiles_pc * C

    src = edge_index[0].astype(np.int64)
    dst = edge_index[1].astype(np.int64)
    tileid = dst // P
    order = np.argsort(tileid, kind="stable")
    ds = dst[order]
    ss = src[order]
    tid_s = tileid[order]

    counts = np.bincount(tileid, minlength=ntiles)
    cpt = counts.reshape(C, ntiles_pc)
    kt = [max(1, int(math.ceil(cpt[:, t].max() / P))) for t in range(ntiles_pc)]
    offs = np.zeros(ntiles_pc + 1, np.int64)
    np.cumsum(kt, out=offs[1:])
    sumkt = int(offs[-1])
    SL = sumkt * P  # slots per core

    tile_start = np.zeros(ntiles + 1, np.int64)
    np.cumsum(counts, out=tile_start[1:])
    rank = np.arange(E, dtype=np.int64) - tile_start[tid_s]
    core = tid_s // ntiles_pc
    t_local = tid_s % ntiles_pc
    slot = offs[t_local] * P + rank  # slot within the core's stream

    nodes16 = _to_fp8(nodes)
    ef16 = _to_fp8(edge_features)

    per_core = []
    for c in range(C):
        m = core == c
        sl_c = slot[m]
        # source rows + edge rows into slot order (pads stay zero)
        ns = np.zeros((SL, D), NP_FP8)
        ns[sl_c] = nodes16[ss[m]]
        ef = np.zeros((SL, D), NP_FP8)
        ef[sl_c] = ef16[order[m]]
        dof = np.full(SL, -1.0, np.float32)
        dof[sl_c] = (ds[m] - (ds[m] // P) * P).astype(np.float32)

        nsT = _blocksT(ns)  # [P, SL]
        efT = _blocksT(ef)  # [P, SL]
        # merged stream: per tile t, kt[t] ns-tiles then kt[t] ef-tiles
        nsef = np.empty((P, 2 * SL), NP_FP8)
        for t in range(ntiles_pc):
            a, b = int(offs[t]) * P, int(offs[t + 1]) * P
            w = b - a
            nsef[:, 2 * a : 2 * a + w] = nsT[:, a:b]
            nsef[:, 2 * a + w : 2 * b] = efT[:, a:b]
        dstoffT = np.ascontiguousarray(
            _trunc_bf16(dof.reshape(sumkt, P).T)
        )  # [P, sumkt]
        per_core.append(dict(nsefT=nsef, dstoffT=dstoffT))
    return kt, per_core


def build_program(D, ntiles_pc, kid, kov, debug=False):
    """Build the SPMD Bass program (identical across cores)."""
    assert D == P
    nc = bacc.Bacc("TRN2", target_bir_lowering=False, debug=False, num_devices=C)
    NP_ = ntiles_pc * P
    kt = [kid[t] + kov[t] for t in range(ntiles_pc)]
    offs = np.zeros(ntiles_pc + 1, np.int64)
    np.cumsum(kt, out=offs[1:])
    sumkt = int(offs[-1])
    ktmax = max(kt)
    ovoffs = np.zeros(ntiles_pc + 1, np.int64)
    np.cumsum(kov, out=ovoffs[1:])
    sumkov = int(ovoffs[-1])
    kovmax = max(1, max(kov))

    d = lambda name, shape, dt=F32, kind="ExternalInput": nc.dram_tensor(
        name, shape, dt, kind=kind
    ).ap()

    nsef = d("nsefT", [P, 2 * sumkt * P], FP8)
    XF = 8 * D + 2 * D + 3
    XB = kovmax * P + P + 3 * D + max(1, sumkov)
    packF = d("packF", [P, XF])
    packB = d("packB", [P, XB], BF16)
    negpad = d("negpad", [P, NP_], BF16)
    ownT_d = d("own_nodesT", [P, NP_])
    out = d("out_own", [P, NP_], kind="ExternalOutput")
    aggdbg = d("aggdbg", [P, ntiles_pc * D], kind="ExternalOutput") if debug else None

    with tile.TileContext(nc) as tc:
        with (
            tc.tile_pool(name="const", bufs=1) as cp,
            tc.tile_pool(name="sb", bufs=3) as sb,
            tc.tile_pool(name="big", bufs=3) as bigp,
            tc.tile_pool(name="psum", bufs=1, space="PSUM") as pp,
            tc.tile_pool(name="psum1", bufs=1, space="PSUM") as pp1,
            tc.tile_pool(name="psum3", bufs=3, space="PSUM") as pp3,
        ):
            packF_s = cp.tile([P, XF], F32, tag="packF")
            nc.sync.dma_start(out=packF_s[:], in_=packF[:])
            packB_s = cp.tile([P, XB], BF16, tag="packB")
            nc.scalar.dma_start(out=packB_s[:], in_=packB[:])
            negpad_s = cp.tile([P, NP_], BF16, tag="negpad")
            nc.scalar.dma_start(out=negpad_s[:], in_=negpad[:])

            mbB_s = packF_s[:, : 8 * D]
            wt_f = packF_s[:, 8 * D : 9 * D]
            wb_f = packF_s[:, 9 * D : 10 * D]
            ub1_s = packF_s[:, 10 * D : 10 * D + 1]
            ub2_s = packF_s[:, 10 * D + 1 : 10 * D + 2]
            mb1c_s = packF_s[:, 10 * D + 2 : 10 * D + 3]
            iotaB_s = packB_s[:, : kovmax * P]
            id16_s = packB_s[:, kovmax * P : kovmax * P + P]
            _b0 = kovmax * P + P
            ua_s = packB_s[:, _b0 : _b0 + D]
            ub_s = packB_s[:, _b0 + D : _b0 + 2 * D]
            uw2_s = packB_s[:, _b0 + 2 * D : _b0 + 3 * D]
            doff_s = packB_s[:, _b0 + 3 * D :]
            wt_s = cp.tile([D, D], FP8, tag="wt8")
            wb_s = cp.tile([D, D], FP8, tag="wb8")
            nc.vector.tensor_scalar(
                out=wt_s[:], in0=wt_f, scalar1=WSCALE, scalar2=None, op0=OP.mult
            )
            nc.vector.tensor_scalar(
                out=wb_s[:], in0=wb_f, scalar1=WSCALE, scalar2=None, op0=OP.mult
            )
            zcol = cp.tile([P, 1], F32, tag="zcol")
            nc.vector.memset(zcol[:], 0)
            siluB_s = cp.tile([P, 1], F32, tag="siluB")
            nc.scalar.activation(
                out=siluB_s[:], in_=zcol[:], func=AF.Silu, bias=mb1c_s
            )
            aggT_all = cp.tile([P, ntiles_pc * D], F32, tag="aggT_all")

            # ---- stage 2: edge pipeline (software-pipelined by 1 chunk) ----
            CH = 8  # edge tiles per PSUM chunk (2 banks)
            chunks = []
            for t in range(ntiles_pc):
                nch = math.ceil(kt[t] / CH)
                for ci in range(nch):
                    chunks.append((t, ci, ci == 0, ci == nch - 1))

            state = {}  # t -> (chunk_tile, paggT, ohT)

            def produce(t, ci, first):
                KT = kt[t]
                if first:
                    W2 = 2 * KT * D
                    ctile = bigp.tile([P, 2 * ktmax * D], FP8, tag="chunk")
                    dma = nc.sync if (t % 2 == 0) else nc.scalar
                    dma.dma_start(
                        out=ctile[:, :W2],
                        in_=nsef[
                            :, 2 * int(offs[t]) * D : 2 * int(offs[t]) * D + W2
                        ],
                    )
                    paggT = pp1.tile([P, D], F32, tag="paggT")
                    KV = kov[t]
                    if KV > 0:
                        ohT = bigp.tile([P, kovmax * P], BF16, tag="ohT")
                        a = int(ovoffs[t])
                        nc.vector.tensor_tensor(
                            out=ohT[:, : KV * P].rearrange(
                                "p (f e) -> p f e", e=P
                            ),
                            in0=doff_s[:, a : a + KV].to_broadcast([P, KV, P]),
                            in1=iotaB_s[:, : KV * P].rearrange(
                                "p (f e) -> p f e", e=P
                            ),
                            op=OP.is_equal,
                        )
                    else:
                        ohT = None
                    state[t] = (ctile, paggT, ohT)
                ctile, _, _ = state[t]
                k0 = ci * CH
                cw = min(CH, KT - k0)
                pmsg = pp3.tile([P, CH * P], F32, tag="pmsg")
                for j in range(cw):
                    k = k0 + j
                    reg = pmsg[:, j * P : (j + 1) * P]
                    nc.tensor.matmul(
                        out=reg, lhsT=ctile[:, k * D : (k + 1) * D], rhs=wt_s[:],
                        start=True, stop=False,
                    )
                    nc.tensor.matmul(
                        out=reg,
                        lhsT=ctile[:, (KT + k) * D : (KT + k + 1) * D],
                        rhs=wb_s[:],
                        start=False, stop=True,
                    )
                return (t, ci, cw, pmsg)

            def consume(pend, last):
                t, ci, cw, pmsg = pend
                KT = kt[t]
                k0 = ci * CH
                W = cw * P
                _, paggT, ohT = state[t]
                mpre = sb.tile([P, CH * P], BF16, tag="mpre")
                nc.vector.scalar_tensor_tensor(
                    out=mpre[:, :W], in0=pmsg[:, :W], scalar=1.0 / WSCALE,
                    in1=mbB_s[:, :W], op0=OP.mult, op1=OP.add,
                )
                msg = sb.tile([P, CH * P], BF16, tag="msg")
                nc.scalar.activation(out=msg[:, :W], in_=mpre[:, :W], func=AF.Silu)
                KI = kid[t]
                for j in range(cw):
                    k = k0 + j
                    # aggT[d, j] += sum_e msg[e, d] * scatter[e, j]
                    rhs = (
                        id16_s
                        if k < KI
                        else ohT[:, (k - KI) * P : (k - KI + 1) * P]
                    )
                    nc.tensor.matmul(
                        out=paggT[:],
                        lhsT=msg[:, j * P : (j + 1) * P],
                        rhs=rhs,
                        start=(k == 0),
                        stop=(last and k == KT - 1),
                        skip_group_check=True,
                    )
                if last:
                    # copy agg^T out while cancelling identity-pad
                    # contributions exactly: agg -= silu(b)[d]*padcnt[t][j]
                    nc.vector.scalar_tensor_tensor(
                        out=aggT_all[:, t * D : (t + 1) * D],
                        in0=negpad_s[:, t * P : (t + 1) * P],
                        scalar=siluB_s[:, :1],
                        in1=paggT[:],
                        op0=OP.mult,
                        op1=OP.add,
                    )

            def stage3_group(g):
                gw = min(4, ntiles_pc - g)
                W = gw * P
                ownT = sb.tile([P, 4 * P], F32, tag="ownT")
                nc.sync.dma_start(
                    out=ownT[:, :W], in_=ownT_d[:, g * P : g * P + W]
                )
                ownT16 = sb.tile([P, 4 * P], BF16, tag="ownT16")
                nc.gpsimd.tensor_copy(out=ownT16[:, :W], in_=ownT[:, :W])
                agg16 = sb.tile([P, 4 * P], BF16, tag="agg16")
                nc.gpsimd.tensor_copy(
                    out=agg16[:, :W], in_=aggT_all[:, g * D : g * D + W]
                )
                ph = pp.tile([P, 4 * P], F32, tag="ph")
                nc.tensor.matmul(
                    out=ph[:, :W], lhsT=ua_s, rhs=ownT16[:, :W],
                    start=True, stop=False,
                )
                nc.tensor.matmul(
                    out=ph[:, :W], lhsT=ub_s, rhs=agg16[:, :W],
                    start=False, stop=True,
                )
                hT = sb.tile([P, 4 * P], BF16, tag="hT")
                nc.scalar.activation(
                    out=hT[:, :W], in_=ph[:, :W], func=AF.Silu, bias=ub1_s
                )
                po = pp.tile([P, 4 * P], F32, tag="ph")
                nc.tensor.matmul(
                    out=po[:, :W], lhsT=uw2_s, rhs=hT[:, :W], start=True, stop=True
                )
                oT = sb.tile([P, 4 * P], F32, tag="oT")
                nc.scalar.activation(
                    out=oT[:, :W], in_=po[:, :W], func=AF.Identity, bias=ub2_s
                )
                nc.gpsimd.tensor_tensor(
                    out=oT[:, :W], in0=oT[:, :W], in1=ownT[:, :W], op=OP.add
                )
                nc.sync.dma_start(
                    out=out[:, g * P : g * P + W], in_=oT[:, :W]
                )

            # ---- main loop: 2-deep chunk pipeline, stage-3 interleaved ----
            from collections import deque

            pending = deque()
            done_tiles = 0

            def after_consume(t, last):
                nonlocal done_tiles
                if last:
                    done_tiles += 1
                    if done_tiles % 4 == 0:
                        stage3_group(done_tiles - 4)

            for t, ci, first, last in chunks:
                cur = produce(t, ci, first)
                pending.append((cur, last))
                if len(pending) > 2:
                    (pc, pl) = pending.popleft()
                    consume(pc, pl)
                    after_consume(pc[0], pl)
            while pending:
                (pc, pl) = pending.popleft()
                consume(pc, pl)
                after_consume(pc[0], pl)
            for g in range(4 * (done_tiles // 4), ntiles_pc, 4):
                stage3_group(g)
            if debug:
                nc.sync.dma_start(out=aggdbg[:], in_=aggT_all[:])

    nc.compile()
    return nc


def _run(nc, in_maps, trace=False):
    return bass_utils.run_bass_kernel_spmd(
        nc, in_maps, core_ids=list(range(C)), trace=trace
    )


def make_in_maps(nodes, edge_index, edge_features, mw1, mb1, uw1, ub1, uw2, ub2,
                 ntiles_pc):
    N, D = nodes.shape
    NP_ = ntiles_pc * P
    N2 = NP_ * C
    kt, per_core = _host_prep(nodes, edge_index, edge_features, ntiles_pc)

    nodes_pad = np.zeros((N2, D), np.float32)
    nodes_pad[:N] = nodes
    kovmax = max(1, max(kov))
    iotaB = np.broadcast_to(
        np.tile(np.arange(P, dtype=np.float32), kovmax), (P, kovmax * P)
    ).astype(np.float32)
    packB_common = np.concatenate(
        [
            _trunc_bf16(iotaB),
            _trunc_bf16(np.eye(P, dtype=np.float32)),
            _trunc_bf16(uw1[:D]),
            _trunc_bf16(uw1[D:]),
            _trunc_bf16(uw2),
        ],
        axis=1,
    )

    packF = np.concatenate(
        [
            np.broadcast_to(np.tile(mb1.astype(np.float32), 8), (P, 8 * D)),
            np.ascontiguousarray(mw1[:D], np.float32).T.T,
            np.ascontiguousarray(mw1[D:], np.float32),
            ub1.reshape(D, 1).astype(np.float32),
            ub2.reshape(D, 1).astype(np.float32),
            _trunc_bf16(mb1.reshape(D, 1)).astype(np.float32),
        ],
        axis=1,
    ).astype(np.float32)
    shared = dict(
        packF=np.ascontiguousarray(packF),
    )
    in_maps = []
    for c in range(C):
        m = dict(shared)
        own = nodes_pad[c * NP_ : (c + 1) * NP_]  # [NP_, D]
        m["own_nodesT"] = np.ascontiguousarray(own.T)  # [P(d), NP_]
        m["nsefT"] = per_core[c]["nsefT"]
        m["dstoffT"] = per_core[c]["dstoffT"]
        in_maps.append(m)
    return kt, in_maps


def kernel(nodes, edge_index, edge_features, mw1, mb1, uw1, ub1, uw2, ub2):
    nodes = np.asarray(nodes, np.float32)
    edge_index = np.asarray(edge_index, np.int32)
    edge_features = np.asarray(edge_features, np.float32)
    N, D = nodes.shape
    ntiles_pc = math.ceil(N / (C * P))
    kt, in_maps = make_in_maps(
        nodes, edge_index, edge_features, mw1, mb1, uw1, ub1, uw2, ub2, ntiles_pc
    )
    nc = build_program(D, ntiles_pc, kt)
    res = _run(nc, in_maps)
    NP_ = ntiles_pc * P
    # out_own is [P(d? no: partition = n%128), ntiles*D] -> rows
    outs = [np.ascontiguousarray(res.results[c]["out_own"].T) for c in range(C)]
    out = np.concatenate(outs, axis=0)
    return out[:N].astype(np.float32)


if __name__ == "__main__":
    rng = np.random.default_rng(0)
    N, E, D = 4096, 16384, 128
    nodes = rng.standard_normal((N, D), dtype=np.float32)
    edge_index = rng.integers(0, N, (2, E)).astype(np.int32)
    ef = rng.standard_normal((E, D), dtype=np.float32)
    s2, s1 = 1 / np.sqrt(2 * D), 1 / np.sqrt(D)
    mw1 = rng.uniform(-s2, s2, (2 * D, D)).astype(np.float32)
    mb1 = rng.uniform(-s2, s2, D).astype(np.float32)
    uw1 = rng.uniform(-s2, s2, (2 * D, D)).astype(np.float32)
    ub1 = rng.uniform(-s2, s2, D).astype(np.float32)
    uw2 = rng.uniform(-s1, s1, (D, D)).astype(np.float32)
    ub2 = rng.uniform(-s1, s1, D).astype(np.float32)

    def silu(x):
        return x / (1 + np.exp(-x))

    def ref():
        src, dst = edge_index
        msg = silu(np.concatenate([nodes[src], ef], 1) @ mw1 + mb1)
        agg = np.zeros((N, D), np.float32)
        np.add.at(agg, dst, msg)
        upd = silu(np.concatenate([nodes, agg], 1) @ uw1 + ub1) @ uw2 + ub2
        return nodes + upd
    out = kernel(nodes, edge_index, ef, mw1, mb1, uw1, ub1, uw2, ub2)
    exp = ref()
    err = np.abs(out - exp).max() / np.abs(exp).max()
    print("tiny rel err:", err)
